# revision 5
# baseline (speedup 1.0000x reference)
"""Causal single-head attention (B=4, S=2048, D=1024, f32) on 8 trn2 cores.

Sharding: data-parallel over batch (4) x 2-way causal-balanced query split.
Core c handles batch b=c//2 and query 128-row blocks {2j+h : j=0..7} where
h=c%2.  A per-core column permutation of x^T (own-parity blocks first)
makes the instruction stream identical on all 8 cores; the residual
h-asymmetry is carried by a per-core 128x128 mask input (m2).

K/V projection dedup vs the all-local version: each core computes K/V only
for its OWN-parity sequence blocks (half the columns) and exchanges the
halves with its pair core over NC-to-NC remote DMA (relative destination
tpb^1, which is the same-SEngine neighbor = the pair core under both the
identity and trn2u logical->physical NC maps).  The exchange is 8 slot-
varied remote_dma_broadcast transfers per tensor so all 16 SDMA engines
carry slices in parallel.  Kernel-entry ordering comes from the prelude
pair-barrier AllGather (bir kernel barrier); data arrival is signaled by
monotonic semaphores that the consuming tensor-engine instructions wait on
(waits attached post-Tile, since Tile's scheduler cannot model remotely-
incremented semaphores).

Only the own half of x^T is shipped to each core (2MB instead of 4MB).

All matmuls run in bf16 (inputs pre-cast/pre-transposed on the host), f32
PSUM accumulation, f32 softmax normalization and f32 output.  Scores are
computed transposed (s^T[k,q]) so that exp goes psum->sbuf on the scalar
engine, p^T is directly the stationary operand of the context matmul, and
row sums come from ones^T @ p^T matmuls accumulated in PSUM.

The final Tile drain / multi-wait instructions are legalized by Bacc's
generate_event_semaphores pass, so the program is built with bacc.Bacc and
finalized before running.
"""

import numpy as np
import ml_dtypes

B, S, D = 4, 2048, 1024
P = 128
DI = D // P          # 8 contraction subtiles
NBLK = S // P        # 16 sequence blocks
NSLOT = 8            # query blocks per core
QCORE = NSLOT * P    # 1024 query rows per core
HALF = QCORE         # 1024 own-parity sequence columns per core
SCALE = 1.0 / 32.0   # 1/sqrt(D)
BF16 = ml_dtypes.bfloat16

_PROGRAM = None


def _slot_kbs(j):
    """Permuted k-block indices slot j attends to (uniform across cores)."""
    return list(range(0, j + 1)) + list(range(NSLOT, NSLOT + j + 1))


def _build_program():
    import concourse.bacc as bacc
    import concourse.mybir as mybir
    import concourse.tile as tile

    dt = mybir.dt
    f32 = dt.float32
    bf = dt.bfloat16
    Exp = mybir.ActivationFunctionType.Exp

    nc = bacc.Bacc("TRN2", monotonic_sem_count=2)
    semK = nc.monotonic_semaphore(0).sem()  # peer kT half arrived
    semV = nc.monotonic_semaphore(1).sem()  # peer v half arrived
    lsem = nc.alloc_semaphore("rdma_local")

    xT = nc.dram_tensor("xT", [D, HALF], bf, kind="ExternalInput")
    wqT = nc.dram_tensor("wqT", [D, D], bf, kind="ExternalInput")
    wkT = nc.dram_tensor("wkT", [D, D], bf, kind="ExternalInput")
    wvT = nc.dram_tensor("wvT", [D, D], bf, kind="ExternalInput")
    tri = nc.dram_tensor("tri", [P, P], bf, kind="ExternalInput")
    m2 = nc.dram_tensor("m2", [P, P], bf, kind="ExternalInput")
    y = nc.dram_tensor("y", [QCORE, D], f32, kind="ExternalOutput")

    trigK = None          # trigger instruction that must sit behind the barrier
    first_score = None    # first PE consumer of peer kT data
    first_ctx = None      # first PE consumer of peer v data

    with tile.TileContext(nc) as tc:
        with tc.tile_pool(name="pers", bufs=1) as pers:
            tri_sb = pers.tile([P, P], bf, tag="tri", name="tri")
            m2_sb = pers.tile([P, P], bf, tag="m2", name="m2")
            ones_sb = pers.tile([P, 1], bf, tag="ones", name="ones")
            nc.vector.memset(ones_sb[:], 1.0)

            # persistent per-core tensors (bf16):
            #   kT_all [o-sub on partitions][half, oi, s]  (half 0 = own)
            #   v_all  [s-in-block on partitions][perm block, o]
            #   qT_o   [o-sub][q]
            kT_all = pers.tile([P, 2, DI, HALF], bf, tag="kT", name="kT")
            v_all = pers.tile([P, NBLK, D], bf, tag="v", name="v")
            qT_o = [pers.tile([P, QCORE], bf, tag=f"qT{oi}", name=f"qT{oi}") for oi in range(DI)]

            def kT_ap(oi, kb, w=P):
                """kT columns for perm k-block kb (w columns starting there)."""
                half, pos = kb // NSLOT, kb % NSLOT
                return kT_all[:, half, oi, pos * P : pos * P + w]

            # ---- load + projections (xT / weights freed afterwards) ----
            # Input DMAs are chunked and ordered by first use (wk halves,
            # then both xT 512-col chunks on the scalar SWDGE ring, rest of
            # wk, then wq, wv) so the PE starts after ~2MB arrives and never
            # stalls on input bandwidth afterwards.
            NSC = HALF // 512
            with (
                tc.tile_pool(name="ld", bufs=1) as ld,
                tc.tile_pool(name="ppsum", bufs=4, space="PSUM") as ppsum,
            ):
                xT4 = xT.rearrange("(di p) (sc s) -> di p sc s", p=P, s=512)
                wq3 = wqT.rearrange("(di p) o -> di p o", p=P)
                wk3 = wkT.rearrange("(di p) o -> di p o", p=P)
                wv3 = wvT.rearrange("(di p) o -> di p o", p=P)
                wk_d = []
                for di in range(DI):
                    t = ld.tile([P, D], bf, tag=f"wk{di}", name=f"wk{di}")
                    nc.sync.dma_start(t[:, 0:512], wk3[di, :, 0:512])
                    wk_d.append(t)
                xT_c = [[None] * NSC for _ in range(DI)]
                for sc in range(NSC):
                    for di in range(DI):
                        t = ld.tile([P, 512], bf, tag=f"xT{di}_{sc}", name=f"xT{di}_{sc}")
                        nc.scalar.dma_start(t[:], xT4[di, :, sc])
                        xT_c[di][sc] = t
                nc.scalar.dma_start(tri_sb[:], tri[:])
                nc.scalar.dma_start(m2_sb[:], m2[:])
                for di in range(DI):
                    nc.sync.dma_start(wk_d[di][:, 512:D], wk3[di, :, 512:D])
                wq_d, wv_d = [], []
                for name, src3, dst in (("wv", wv3, wv_d), ("wq", wq3, wq_d)):
                    for di in range(DI):
                        t = ld.tile([P, D], bf, tag=f"{name}{di}", name=f"{name}{di}")
                        nc.sync.dma_start(t[:], src3[di])
                        dst.append(t)

                def proj(lhs_fn, rhs_fn, out_ap, n):
                    ps = ppsum.tile([P, n], f32, tag="ppsum", name="ppsum")
                    for di in range(DI):
                        nc.tensor.matmul(
                            ps[:],
                            lhs_fn(di),
                            rhs_fn(di),
                            start=(di == 0),
                            stop=(di == DI - 1),
                        )
                    nc.vector.tensor_copy(out_ap, ps[:])

                for sc in range(NSC):  # kT own half: [o, s own], s-chunk outer
                    cols = slice(sc * 512, (sc + 1) * 512)
                    for oi in range(DI):
                        oc = slice(oi * P, (oi + 1) * P)
                        proj(
                            lambda di, oc=oc: wk_d[di][:, oc],
                            lambda di, sc=sc: xT_c[di][sc][:],
                            kT_all[:, 0, oi, cols], 512,
                        )
                # exchange kT halves with the pair core (slot-varied so each
                # transfer rides a distinct pair of SDMA engines)
                for k in range(NSLOT):
                    rdests = [None] * 8
                    rdests[k] = (0, 1)
                    nc.gpsimd.remote_dma_broadcast(
                        kT_all[:, 1, k, :], kT_all[:, 0, k, :],
                        remote_sem=semK, local_sem=lsem, rdests=rdests,
                    )
                trigK = nc.gpsimd.trigger_dma(count=None)

                for si in range(NSLOT):  # v own half: [s, o] per perm block
                    sc, lo = si // 4, (si % 4) * P
                    for oh in range(D // 512):
                        cols = slice(oh * 512, (oh + 1) * 512)
                        proj(
                            lambda di, sc=sc, lo=lo: xT_c[di][sc][:, lo : lo + P],
                            lambda di, cols=cols: wv_d[di][:, cols],
                            v_all[:, si, cols], 512,
                        )
                for k in range(NSLOT):
                    rdests = [None] * 8
                    rdests[k] = (0, 1)
                    nc.gpsimd.remote_dma_broadcast(
                        v_all[:, NSLOT + k, :], v_all[:, k, :],
                        remote_sem=semV, local_sem=lsem, rdests=rdests,
                    )
                nc.gpsimd.trigger_dma(count=None)

                for qc in range(QCORE // 512):  # qT: [o, q] own rows
                    cols = slice(qc * 512, (qc + 1) * 512)
                    for oi in range(DI):
                        oc = slice(oi * P, (oi + 1) * P)
                        proj(
                            lambda di, oc=oc: wq_d[di][:, oc],
                            lambda di, qc=qc: xT_c[di][qc][:],
                            qT_o[oi][:, cols], 512,
                        )

            # ---- attention ----
            pT = [pers.tile([P, QCORE], bf, tag=f"pT{kb}", name=f"pT{kb}") for kb in range(NBLK)]
            with (
                tc.tile_pool(name="spsum", bufs=2, space="PSUM") as spsum,
                tc.tile_pool(name="rpsum", bufs=2, space="PSUM") as rpsum,
                tc.tile_pool(name="cpsum", bufs=4, space="PSUM") as cpsum,
                tc.tile_pool(name="small", bufs=2) as small,
                tc.tile_pool(name="reciprocals", bufs=1) as rpool,
            ):
                for c in range(2):  # per 512 q: scores in 256-wide chunks
                    for c2 in (2 * c, 2 * c + 1):  # slots {2*c2, 2*c2+1}
                        cols = slice(c2 * 256, (c2 + 1) * 256)
                        for kb in _slot_kbs(2 * c2 + 1):
                            ps = spsum.tile([P, 256], f32, tag="spsum", name="spsum")
                            for oi in range(DI):
                                mm = nc.tensor.matmul(
                                    ps[:],
                                    kT_ap(oi, kb),
                                    qT_o[oi][:, cols],
                                    start=(oi == 0),
                                    stop=(oi == DI - 1),
                                )
                                if first_score is None:
                                    first_score = mm
                            nc.scalar.activation(
                                pT[kb][:, cols], ps[:], Exp, scale=SCALE
                            )
                        # boundary masks (multiplicative, post-exp)
                        for j in (2 * c2, 2 * c2 + 1):
                            qc = slice(j * P, (j + 1) * P)
                            nc.vector.tensor_mul(pT[j][:, qc], pT[j][:, qc], tri_sb[:])
                            nc.vector.tensor_mul(
                                pT[NSLOT + j][:, qc], pT[NSLOT + j][:, qc], m2_sb[:]
                            )
                    # row sums (pT.T @ ones -> [128,1] psum, q on partitions)
                    # and context: ctx[q, o] = sum_k p^T[k,q] * v[k,o]
                    for j in range(4 * c, 4 * c + 4):
                        qc = slice(j * P, (j + 1) * P)
                        kbs = _slot_kbs(j)
                        rsp = rpsum.tile([P, 1], f32, tag="rsp", name="rsp")
                        for i, kb in enumerate(kbs):
                            nc.tensor.matmul(
                                rsp[:],
                                pT[kb][:, qc],
                                ones_sb[:, 0:1],
                                start=(i == 0),
                                stop=(i == len(kbs) - 1),
                            )
                        recip = rpool.tile([P, 1], f32, tag=f"recip{j}", name=f"recip{j}")
                        nc.vector.reciprocal(recip[:], rsp[:])
                        for oh in range(D // 512):
                            ocols = slice(oh * 512, (oh + 1) * 512)
                            cps = cpsum.tile([P, 512], f32, tag="cpsum", name="cpsum")
                            for i, kb in enumerate(kbs):
                                mm = nc.tensor.matmul(
                                    cps[:],
                                    pT[kb][:, qc],
                                    v_all[:, kb, ocols],
                                    start=(i == 0),
                                    stop=(i == len(kbs) - 1),
                                )
                                if first_ctx is None:
                                    first_ctx = mm
                            ct = small.tile([P, 512], f32, tag="ct", name="ct")
                            nc.vector.tensor_scalar_mul(ct[:], cps[:], recip[:, 0:1])
                            nc.sync.dma_start(y[qc, ocols], ct[:])

    # ---- post-Tile: external synchronization ----
    # Tile's scheduling simulator cannot model semaphore increments that
    # arrive from the pair core, so those waits are attached here and
    # legalized by Bacc's passes during finalize().
    def attach_wait(binst, sem, value):
        ins = binst.ins
        w = mybir.SyncWait(
            sync_type="semaphore", id=sem.num, ant_name=sem.name,
            wait_mode="sem-ge-imm", wait_value=value,
        )
        old = ins.sync_info
        if old is None:
            ins.sync_info = mybir.SyncInfo(on_wait=[w], on_update=[])
        else:
            ins.sync_info = mybir.SyncInfo(
                on_wait=[*old.on_wait, w], on_update=old.on_update
            )

    # pair-wise kernel-entry barrier: the first trigger may only fire once
    # the pair core has entered the kernel (its semaphores are cleared and
    # its SBUF layout is live).  Registering the groups here makes Bacc
    # insert the prelude AllGather that bumps the barrier semaphore.
    groups = [[0, 1], [2, 3], [4, 5], [6, 7]]
    nc._bir_kernel_barrier_sem_replica_groups.extend(set(g) for g in groups)
    bsem = nc._bir_kernel_barrier_sem
    attach_wait(trigK, bsem, nc.bir_kernel_barrier_sem_inc)
    # each of the 8 broadcasts bumps the remote semaphore by 16//8 = 2 on
    # arrival of all bytes
    attach_wait(first_score, semK, 16)
    attach_wait(first_ctx, semV, 16)

    nc.finalize()
    return nc


def _get_program():
    global _PROGRAM
    if _PROGRAM is None:
        _PROGRAM = _build_program()
    return _PROGRAM


def _host_prep(x, Wq, Wk, Wv):
    """Per-core input maps: transposed/cast weights and per-core permuted,
    own-parity-half x^T."""
    x = np.asarray(x, dtype=np.float32)
    tri_np = (np.arange(P)[None, :] >= np.arange(P)[:, None]).astype(BF16)
    masks = {0: np.zeros((P, P), dtype=BF16), 1: np.ones((P, P), dtype=BF16)}
    wqT = np.ascontiguousarray(np.asarray(Wq, dtype=np.float32).T).astype(BF16)
    wkT = np.ascontiguousarray(np.asarray(Wk, dtype=np.float32).T).astype(BF16)
    wvT = np.ascontiguousarray(np.asarray(Wv, dtype=np.float32).T).astype(BF16)
    in_maps = []
    for c in range(8):
        b, h = c // 2, c % 2
        perm = [2 * j + h for j in range(NSLOT)]
        xTb = np.asarray(x[b]).T.reshape(D, NBLK, P)[:, perm, :].reshape(D, HALF)
        in_maps.append(
            {
                "xT": np.ascontiguousarray(xTb).astype(BF16),
                "wqT": wqT,
                "wkT": wkT,
                "wvT": wvT,
                "tri": tri_np,
                "m2": masks[h],
            }
        )
    return in_maps


def run(x, Wq, Wk, Wv, **spmd_kwargs):
    """Run on all 8 cores; returns (out [B,S,D] f32, BassKernelResults)."""
    from concourse.bass_utils import run_bass_kernel_spmd

    nc = _get_program()
    in_maps = _host_prep(x, Wq, Wk, Wv)
    res = run_bass_kernel_spmd(nc, in_maps, core_ids=list(range(8)), **spmd_kwargs)
    out = np.empty((B, S, D), dtype=np.float32)
    for c in range(8):
        b, h = c // 2, c % 2
        yc = res.results[c]["y"]
        for j in range(NSLOT):
            g = 2 * j + h
            out[b, g * P : (g + 1) * P, :] = yc[j * P : (j + 1) * P, :]
    return out, res


def kernel(x, Wq, Wk, Wv):
    out, _ = run(x, Wq, Wk, Wv)
    return out
